# revision 10
# baseline (speedup 1.0000x reference)
"""Trainium2 Bass kernel for nn_NeuralQKM: K[i,j] = |<psi_i|psi_j>|^2.

Math. The circuit's only per-sample gates are last-layer RY rotations, so
S[b] = (prod_q RY_q^T(X[b,q])) psi' with psi' fixed (all shared gates; the
final CNOT chain is a common permutation and drops out of the Gram).
Expanding the tensor-product rotation in the product-feature basis
Phi_b[u] = prod_q (cos(X/2) if u_q=0 else sin(X/2)):

    S[b,j] = sum_u Phi_b[u] * (-1)^{|j&u|} * psi'[j^u]

Split psi' = psi'_0 e_0 + r (||r|| ~ 0.04 since params are tiny):

    S = psi'_0 * (sgn . Phi)  +  Phi @ W_r,   W_r[u,j] = (-1)^{|j&u|} r[j^u]

The main term is exact host math (O(B*DIM)); only the small tail needs a
device matmul, which tolerates fp8.

Device pass 1 (state-sharded): T^T = W_r^T Phi^T via fp8e4m3 DoubleRow
matmuls (K=256/instruction at 0.5 cycles/row). Core r computes 512 states x
4096 samples. Host assembles S = main + tail, normalizes per sample,
quantizes planes A=Re(S), B=Im(S), P=fp8(A+B), M=fp8(A-B) at scale LAM.

Device pass 2 (row-sharded, block-cyclic symmetric): 3-product Karatsuba
Gram in fp8 DoubleRow: M1 = A_r A_c^T, M2 = B_r B_c^T,
M3 = (A_r+B_r)(A_c-B_c)^T; Gre = M1+M2, -Gim = M1-M2-M3. Post-ops apply a
per-state norm correction K = (Gre^2+Gim^2)/(rho_i^2 rho_j^2) with
rho^2 = ||quantized state||^2 (host-known), which cancels the dominant fp8
quantization error on the large entries of K. Output per core is the
transposed block strip K[rows, cols].T in bf16; host mirrors the symmetric
blocks.
"""
import numpy as np
import ml_dtypes
import orjson

import concourse.bass as bass
import concourse.mybir as mybir
import concourse.tile as tile
from concourse.bass_utils import run_bass_kernel_spmd

N_QUBITS = 12
N_LAYERS = 5
DIM = 2 ** N_QUBITS          # 4096
B = 4096
NCORES = 8
BLK = B // NCORES            # 512 rows per core in pass 2
NDBLK = 5                    # diagonal + 4 off-diagonal column blocks
NB_COLS = NDBLK * BLK        # 2560 rhs columns per core
NBLK = NB_COLS // 128        # 20 column blocks of 128
KCH = DIM // 256             # 16 contraction chunks of K=256 (DoubleRow)
LAM = 64.0                   # fp8 quantization scale for state planes

f32 = mybir.dt.float32
f8 = mybir.dt.float8e4
bf16 = mybir.dt.bfloat16
npf8 = ml_dtypes.float8_e4m3
npbf = ml_dtypes.bfloat16

# ----------------------------------------------------------------------------
# walrus in this toolchain rejects >1 sync-wait per instruction; Tile emits
# several. Engines are serial, so an extra wait is equivalent to a standalone
# EventSemaphore wait right before the instruction on the same engine.
# ----------------------------------------------------------------------------


def _legalize_multiwait_json(bir: bytes) -> bytes:
    m = orjson.loads(bir)
    changed = False
    for func in m.get("functions", []):
        for blk in func.get("blocks", []):
            out = []
            for inst in blk.get("instructions", []):
                sync = inst.get("sync_info")
                waits = (sync or {}).get("on_wait") or []
                if len(waits) > 1:
                    changed = True
                    for i, w in enumerate(waits[:-1]):
                        out.append({
                            "debug": inst.get("debug", 0),
                            "engine": inst["engine"],
                            "ins": [],
                            "name": f"{inst['name']}-xw{i}",
                            "opcode": "EventSemaphore",
                            "outs": [],
                            "sync_info": {"on_update": [], "on_wait": [w]},
                        })
                    sync["on_wait"] = [waits[-1]]
                out.append(inst)
            blk["instructions"] = out
    return orjson.dumps(m) if changed else bir


_patched = False


def _install_waitfix():
    global _patched
    if _patched:
        return
    _patched = True
    orig = bass.Bass.to_json_bytes

    def patched(self):
        return _legalize_multiwait_json(orig(self))

    bass.Bass.to_json_bytes = patched


# ----------------------------------------------------------------------------
# Host math: psi' (state after all shared circuit parts), complex64 to track
# the reference's precision.
# ----------------------------------------------------------------------------


def _host_psi(params: np.ndarray) -> np.ndarray:
    params = np.asarray(params, np.float32)
    psi = np.zeros(DIM, np.complex64)
    psi[0] = 1.0
    for l in range(N_LAYERS):
        for q in range(N_QUBITS):
            phi, theta, lam = (np.complex64(params[l, q, i]) for i in range(3))
            rz_p = np.array([[np.exp(-0.5j * phi), 0], [0, np.exp(0.5j * phi)]],
                            np.complex64)
            rz_l = np.array([[np.exp(-0.5j * lam), 0], [0, np.exp(0.5j * lam)]],
                            np.complex64)
            c, s = np.cos(0.5 * theta), np.sin(0.5 * theta)
            ry = np.array([[c, -s], [s, c]], np.complex64)
            U = rz_l @ ry @ rz_p
            # reference einsum applies U^T
            st = psi.reshape(2 ** q, 2, -1)
            psi = np.einsum("st,lsr->ltr", U, st).astype(np.complex64).reshape(-1)
        if l < N_LAYERS - 1:
            for q in range(N_QUBITS - 1):
                st = psi.reshape(2 ** q, 2, 2, -1)
                st = np.stack([st[:, 0], np.flip(st[:, 1], axis=1)], axis=1)
                psi = st.reshape(-1)
    return psi


def _popcount_sign() -> np.ndarray:
    j = np.arange(DIM)
    pop = np.zeros(DIM, np.int64)
    for q in range(N_QUBITS):
        pop += (j >> q) & 1
    return np.where(pop % 2 == 0, 1.0, -1.0).astype(np.float32)


def _features(X: np.ndarray) -> np.ndarray:
    """Phi[b, u] = prod_q (cos(X/2) if bit(11-q) of u is 0 else sin(X/2))."""
    c = np.cos(0.5 * X).astype(np.float32)
    s = np.sin(0.5 * X).astype(np.float32)
    phi = np.ones((B, 1), np.float32)
    for q in range(N_QUBITS):
        phi = np.stack([phi * c[:, q:q + 1], phi * s[:, q:q + 1]],
                       axis=2).reshape(B, -1)
    return phi


# ----------------------------------------------------------------------------
# Pass 1: tail states T^T = W_r^T Phi^T, fp8 DoubleRow.
# Core r computes states [512r, 512r+512) x all 4096 samples.
# ----------------------------------------------------------------------------


def _build_pass1() -> bass.Bass:
    nc = bass.Bass("TRN2", target_bir_lowering=False, debug=False,
                   num_devices=NCORES)
    # w8[p, pl, kc, i, blk, c] = plane pl of W_r[kc*256+i*128+p, 512r+blk*128+c]
    w_d = nc.dram_tensor("w8", [128, 2, KCH, 2, 4, 128], f8,
                         kind="ExternalInput").ap()
    # phi[n, p, kc, i, b] = Phi8^T[kc*256+i*128+p, n*512+b]
    phi_d = nc.dram_tensor("phi", [8, 128, KCH, 2, 512], f8,
                           kind="ExternalInput").ap()
    # t[n, pl, blk, p, b] = lamP*lamW * T^T[pl, 512r+blk*128+p, n*512+b]
    t_d = nc.dram_tensor("t", [8, 2, 4, 128, 512], bf16,
                         kind="ExternalOutput").ap()

    with tile.TileContext(nc) as tc:
        with (
            tc.tile_pool(name="wpool", bufs=1) as wpool,
            tc.tile_pool(name="ppool", bufs=2) as phipool,
            tc.tile_pool(name="spool", bufs=3) as spool,
            tc.tile_pool(name="psum", bufs=1, space="PSUM") as psum,
        ):
            w8 = wpool.tile([128, 2, KCH, 2, 4, 128], f8, tag="w8")

            for n in range(8):
                phi = phipool.tile([128, KCH, 2, 512], f8, tag="phi")
                if n == 0:
                    # fine-grained opening stream: interleave phi0 and w8
                    # k-pieces so the first matmuls start after ~2 small DMAs
                    # instead of after three full-tile transfers
                    for k in range(KCH):
                        nc.sync.dma_start(phi[:, k], phi_d[0, :, k])
                        nc.sync.dma_start(w8[:, 0, k], w_d[:, 0, k])
                        nc.sync.dma_start(w8[:, 1, k], w_d[:, 1, k])
                else:
                    nc.sync.dma_start(phi[:], phi_d[n])
                for pl in range(2):
                    for blk in range(4):
                        ps = psum.tile([128, 512], f32, tag=f"ps{pl}{blk}",
                                       name=f"ps_{n}_{pl}_{blk}")
                        for k in range(KCH):
                            nc.tensor.matmul(
                                ps[:], w8[:, pl, k, :, blk, :], phi[:, k],
                                start=(k == 0), stop=(k == KCH - 1),
                                perf_mode=mybir.MatmulPerfMode.DoubleRow)
                        st = spool.tile([128, 512], bf16, tag=f"st{pl}{blk}",
                                        name=f"st_{n}_{pl}_{blk}")
                        # gpsimd cannot access PSUM; alternate DVE/ACT
                        if (pl * 4 + blk) % 2 == 0:
                            nc.vector.tensor_copy(st[:], ps[:])
                        else:
                            nc.scalar.copy(st[:], ps[:])
                        nc.sync.dma_start(t_d[n, pl, blk], st[:])
    return nc


# ----------------------------------------------------------------------------
# Pass 2: Karatsuba Gram + norm-corrected |.|^2, fp8 DoubleRow.
# ----------------------------------------------------------------------------


def _build_pass2() -> bass.Bass:
    nc = bass.Bass("TRN2", target_bir_lowering=False, debug=False,
                   num_devices=NCORES)
    # mv[p, pl, kc, i, f]: planes (A, B, P=A+B) of own rows (moving operand)
    mv_d = nc.dram_tensor("mv8", [128, 3, KCH, 2, BLK], f8,
                          kind="ExternalInput").ap()
    # wt[n, p, pl, kc, i, c]: planes (A, B, M=A-B) of col block n (stationary)
    wt_d = nc.dram_tensor("wt8", [NBLK, 128, 3, KCH, 2, 128], f8,
                          kind="ExternalInput").ap()
    sig_d = nc.dram_tensor("sig", [128, NBLK], f32, kind="ExternalInput").ap()
    wrow_d = nc.dram_tensor("wrow", [1, BLK], f32, kind="ExternalInput").ap()
    ko_d = nc.dram_tensor("ko", [NBLK, 128, BLK], bf16,
                          kind="ExternalOutput").ap()

    with tile.TileContext(nc) as tc:
        with (
            tc.tile_pool(name="mv", bufs=1) as mpool,
            tc.tile_pool(name="wt", bufs=3) as wpool,
            tc.tile_pool(name="dwt", bufs=1) as dpool,
            tc.tile_pool(name="post", bufs=2) as qpool,
            tc.tile_pool(name="psum", bufs=2, space="PSUM") as ppool,
        ):
            sig = mpool.tile([128, NBLK], f32, tag="sig")
            nc.sync.dma_start(sig[:], sig_d)
            wrow = mpool.tile([128, BLK], f32, tag="wrow")
            nc.sync.dma_start(wrow[:], wrow_d[0].partition_broadcast(128))

            mv = mpool.tile([128, 3, KCH, 2, BLK], f8, tag="mv")
            wt0 = wpool.tile([128, 3, KCH, 2, 128], f8, tag="wt", name="wt_0")
            # fine-grained opening: block 0's operands stream plane-by-plane
            # (wt) and quarter-by-quarter (mv) so its matmuls start early
            for pl in range(3):
                nc.sync.dma_start(wt0[:, pl], wt_d[0, :, pl])
                for q in range(4):
                    nc.sync.dma_start(mv[:, pl, 4 * q:4 * q + 4],
                                      mv_d[:, pl, 4 * q:4 * q + 4])

            # blocks 16..19 are this core's diagonal cols = its own rows:
            # planes A, B are copied in-SBUF from mv (saves wt DMA); only the
            # (A-B) plane is loaded. Emitted early: they run during the
            # off-diagonal blocks when the copy engines have slack.
            def _copy(eng, dst, src):
                if eng is nc.scalar:
                    nc.scalar.copy(dst, src)
                else:
                    eng.tensor_copy(dst, src)

            dwts = []
            for d in range(4):
                dwt = dpool.tile([128, 3, KCH, 2, 128], f8, tag=f"dwt{d}")
                csl = slice(128 * d, 128 * (d + 1))
                _copy((nc.gpsimd, nc.scalar, nc.vector, nc.gpsimd)[d],
                      dwt[:, 0], mv[:, 0, :, :, csl])
                _copy((nc.scalar, nc.vector, nc.gpsimd, nc.scalar)[d],
                      dwt[:, 1], mv[:, 1, :, :, csl])
                nc.sync.dma_start(dwt[:, 2], wt_d[16 + d, :, 2])
                dwts.append(dwt)

            for n in range(NBLK):
                if n == 0:
                    wt = wt0
                elif n >= 16:
                    wt = dwts[n - 16]
                else:
                    wt = wpool.tile([128, 3, KCH, 2, 128], f8, tag="wt",
                                    name=f"wt_{n}")
                    nc.sync.dma_start(wt[:], wt_d[n])

                ms = []
                for prod in range(3):
                    ps = ppool.tile([128, BLK], f32, tag=f"m{prod}",
                                    name=f"m{prod}_{n}")
                    for k in range(KCH):
                        nc.tensor.matmul(
                            ps[:], wt[:, prod, k], mv[:, prod, k],
                            start=(k == 0), stop=(k == KCH - 1),
                            perf_mode=mybir.MatmulPerfMode.DoubleRow)
                    ms.append(ps)
                m1, m2, m3 = ms

                # only one PSUM operand allowed per instruction
                c2 = qpool.tile([128, BLK], f32, tag="c2")
                nc.scalar.copy(c2[:], m2[:])
                t1 = qpool.tile([128, BLK], f32, tag="t1")
                nc.vector.tensor_tensor(t1[:], m1[:], c2[:],
                                        mybir.AluOpType.add)
                t2 = qpool.tile([128, BLK], f32, tag="t2")
                nc.vector.tensor_tensor(t2[:], m1[:], c2[:],
                                        mybir.AluOpType.subtract)
                t3 = qpool.tile([128, BLK], f32, tag="t3")
                # gpsimd cannot access PSUM -> DVE for the m3 read
                nc.vector.scalar_tensor_tensor(t3[:], m3[:], -1.0, t2[:],
                                               mybir.AluOpType.mult,
                                               mybir.AluOpType.add)
                sq1 = qpool.tile([128, BLK], f32, tag="sq1")
                nc.scalar.activation(sq1[:], t1[:],
                                     mybir.ActivationFunctionType.Square,
                                     scale=sig[:, n:n + 1])
                sq3 = qpool.tile([128, BLK], f32, tag="sq3")
                nc.scalar.activation(sq3[:], t3[:],
                                     mybir.ActivationFunctionType.Square,
                                     scale=sig[:, n:n + 1])
                ss = qpool.tile([128, BLK], f32, tag="ss")
                nc.gpsimd.tensor_tensor(ss[:], sq1[:], sq3[:],
                                        mybir.AluOpType.add)
                ko = qpool.tile([128, BLK], bf16, tag="ko")
                nc.vector.tensor_tensor(ko[:], ss[:], wrow[:],
                                        mybir.AluOpType.mult)
                nc.sync.dma_start(ko_d[n], ko[:])
    return nc


_nc1 = None
_nc2 = None

PROFILE = False
LAST_PROFILE: dict = {}


def kernel(X: np.ndarray, params: np.ndarray) -> np.ndarray:
    global _nc1, _nc2
    _install_waitfix()
    X = np.asarray(X, np.float32)
    params = np.asarray(params, np.float32)

    # ---- host precompute -------------------------------------------------
    psi = _host_psi(params)
    psi0 = psi[0]
    r = psi.copy()
    r[0] = 0.0
    sgn = _popcount_sign()
    phi = _features(X)                       # (B, DIM) f32

    jj = np.arange(DIM)
    XORm = np.bitwise_xor.outer(jj, jj)      # (u, j)
    ANDm = np.bitwise_and.outer(jj, jj)
    sgn_uj = sgn[ANDm]
    w_re = sgn_uj * r.real[XORm]
    w_im = sgn_uj * r.imag[XORm]
    lam_w = float(224.0 / max(np.abs(w_re).max(), np.abs(w_im).max(), 1e-30))
    w8 = np.stack([(w_re * lam_w).astype(npf8),
                   (w_im * lam_w).astype(npf8)])      # (2, DIM u, DIM j)
    lam_p = 64.0
    phi8t = np.ascontiguousarray((phi.T * lam_p).astype(npf8))   # (u, b)

    # per-core pass-1 inputs
    phi_in = np.ascontiguousarray(
        phi8t.reshape(KCH, 2, 128, 8, 512).transpose(3, 2, 0, 1, 4))
    in_maps1 = []
    for cr in range(NCORES):
        wc = w8[:, :, cr * BLK:(cr + 1) * BLK]        # (2, DIM, 512)
        wc = wc.reshape(2, KCH, 2, 128, 4, 128).transpose(3, 0, 1, 2, 4, 5)
        in_maps1.append({"w8": np.ascontiguousarray(wc), "phi": phi_in})

    if _nc1 is None:
        _nc1 = _build_pass1()
    res1 = run_bass_kernel_spmd(_nc1, in_maps1, core_ids=list(range(NCORES)))

    # ---- host mid: assemble S, quantize planes ---------------------------
    inv_lw = 1.0 / (lam_p * lam_w)
    phiT = phi.T                                      # (j, b)
    A = np.empty((DIM, B), np.float32)
    Bp = np.empty((DIM, B), np.float32)
    for cr in range(NCORES):
        t = res1.results[cr]["t"].astype(np.float32) * inv_lw  # (8,2,4,128,512)
        rows = slice(cr * BLK, (cr + 1) * BLK)
        tt = t.transpose(1, 2, 3, 0, 4).reshape(2, BLK, B)
        A[rows] = tt[0]
        Bp[rows] = tt[1]
    A += psi0.real * sgn[:, None] * phiT
    Bp += psi0.imag * sgn[:, None] * phiT
    nrm = np.sqrt(np.einsum("jb,jb->b", A, A) + np.einsum("jb,jb->b", Bp, Bp))
    A *= (1.0 / nrm)[None, :]
    Bp *= (1.0 / nrm)[None, :]

    A8 = (A * LAM).astype(npf8)
    B8 = (Bp * LAM).astype(npf8)
    A8f = A8.astype(np.float32)
    B8f = B8.astype(np.float32)
    P8 = (A8f + B8f).astype(npf8)
    M8 = (A8f - B8f).astype(npf8)
    rho2 = (np.einsum("jb,jb->b", A8f, A8f)
            + np.einsum("jb,jb->b", B8f, B8f)) / (LAM * LAM)    # (B,)

    pl_mv = np.stack([A8, B8, P8])    # (3, j, b)
    pl_wt = np.stack([A8, B8, M8])
    sig_all = (1.0 / (LAM * LAM * np.sqrt(rho2))).astype(np.float32)
    wrow_all = (1.0 / rho2).astype(np.float32)

    # strip layout: 16 off-diagonal col blocks first (strip offsets
    # 512..2560), the 4 diagonal blocks (offsets 0..512) last — the device
    # fills the diagonal stationary tiles by SBUF copies from mv.
    colrel = np.concatenate([np.arange(BLK, NB_COLS), np.arange(0, BLK)])
    in_maps2 = []
    for cr in range(NCORES):
        cols = (cr * BLK + colrel) % B
        mvc = pl_mv[:, :, cr * BLK:(cr + 1) * BLK]    # (3, DIM, 512)
        mvc = mvc.reshape(3, KCH, 2, 128, BLK).transpose(3, 0, 1, 2, 4)
        wtc = pl_wt[:, :, cols]                       # (3, DIM, 2560)
        wtc = (wtc.reshape(3, KCH, 2, 128, NBLK, 128)
               .transpose(4, 3, 0, 1, 2, 5))
        sig = sig_all[cols].reshape(NBLK, 128).T      # (128, NBLK)
        wrow = wrow_all[cr * BLK:(cr + 1) * BLK][None, :]
        in_maps2.append({
            "mv8": np.ascontiguousarray(mvc),
            "wt8": np.ascontiguousarray(wtc),
            "sig": np.ascontiguousarray(sig),
            "wrow": np.ascontiguousarray(wrow),
        })

    if _nc2 is None:
        _nc2 = _build_pass2()
    res2 = run_bass_kernel_spmd(_nc2, in_maps2, core_ids=list(range(NCORES)))

    # ---- assemble K (with symmetric mirroring) ---------------------------
    K = np.empty((B, B), np.float32)
    for cr in range(NCORES):
        ko = res2.results[cr]["ko"].astype(np.float32)  # (NBLK, 128, BLK)
        rows = slice(cr * BLK, (cr + 1) * BLK)
        for n in range(NBLK):
            gs = (cr * BLK + int(colrel[n * 128])) % B
            colsl = slice(gs, gs + 128)
            K[rows, colsl] = ko[n].T
            d = 1 + n // 4 if n < 16 else 0
            if 0 < d < 4 or (d == 4 and cr < 4):
                K[colsl, rows] = ko[n]
    return K


# revision 19
# speedup vs baseline: 1.0728x; 1.0728x over previous
"""Trainium2 Bass kernel for nn_NeuralQKM: K[i,j] = |<psi_i|psi_j>|^2.

Math. The circuit's only per-sample gates are last-layer RY rotations, so
S[b] = (prod_q RY_q^T(X[b,q])) psi' with psi' fixed (all shared gates; the
final CNOT chain is a common permutation and drops out of the Gram).
Expanding the tensor-product rotation in the product-feature basis
Phi_b[u] = prod_q (cos(X/2) if u_q=0 else sin(X/2)):

    S[b,j] = sum_u Phi_b[u] * (-1)^{|j&u|} * psi'[j^u]

Split psi' = psi'_0 e_0 + r (||r|| ~ 0.04 since params are tiny):

    S = psi'_0 * (sgn . Phi)  +  Phi @ W_r,   W_r[u,j] = (-1)^{|j&u|} r[j^u]

The main term is exact host math (O(B*DIM)); only the small tail needs a
device matmul, which tolerates fp8.

Device pass 1 (state-sharded): T^T = W_r^T Phi^T via fp8e4m3 DoubleRow
matmuls (K=256/instruction at 0.5 cycles/row). Core r computes 512 states x
4096 samples. Host assembles S = main + tail, normalizes per sample,
quantizes planes A=Re(S), B=Im(S), P=fp8(A+B), M=fp8(A-B) at scale LAM.

Device pass 2 (row-sharded, block-cyclic symmetric): 3-product Karatsuba
Gram in fp8 DoubleRow: M1 = A_r A_c^T, M2 = B_r B_c^T,
M3 = (A_r+B_r)(A_c-B_c)^T; Gre = M1+M2, -Gim = M1-M2-M3. Post-ops apply a
per-state norm correction K = (Gre^2+Gim^2)/(rho_i^2 rho_j^2) with
rho^2 = ||quantized state||^2 (host-known), which cancels the dominant fp8
quantization error on the large entries of K. Output per core is the
transposed block strip K[rows, cols].T in bf16; host mirrors the symmetric
blocks.
"""
import numpy as np
import ml_dtypes
import orjson

import concourse.bass as bass
import concourse.mybir as mybir
import concourse.tile as tile
from concourse.bass_utils import run_bass_kernel_spmd

N_QUBITS = 12
N_LAYERS = 5
DIM = 2 ** N_QUBITS          # 4096
B = 4096
NCORES = 8
BLK = B // NCORES            # 512 rows per core in pass 2
NDBLK = 5                    # diagonal + 4 off-diagonal column blocks
NB_COLS = NDBLK * BLK        # 2560 rhs columns per core
NBLK = NB_COLS // 128        # 20 column blocks of 128
KCH = DIM // 256             # 16 contraction chunks of K=256 (DoubleRow)
LAM = 64.0                   # fp8 quantization scale for state planes

f32 = mybir.dt.float32
f8 = mybir.dt.float8e4
bf16 = mybir.dt.bfloat16
npf8 = ml_dtypes.float8_e4m3
npbf = ml_dtypes.bfloat16

# ----------------------------------------------------------------------------
# walrus in this toolchain rejects >1 sync-wait per instruction; Tile emits
# several. Engines are serial, so an extra wait is equivalent to a standalone
# EventSemaphore wait right before the instruction on the same engine.
# ----------------------------------------------------------------------------


def _legalize_multiwait_json(bir: bytes) -> bytes:
    m = orjson.loads(bir)
    changed = False
    for func in m.get("functions", []):
        for blk in func.get("blocks", []):
            out = []
            for inst in blk.get("instructions", []):
                sync = inst.get("sync_info")
                waits = (sync or {}).get("on_wait") or []
                if len(waits) > 1:
                    changed = True
                    for i, w in enumerate(waits[:-1]):
                        out.append({
                            "debug": inst.get("debug", 0),
                            "engine": inst["engine"],
                            "ins": [],
                            "name": f"{inst['name']}-xw{i}",
                            "opcode": "EventSemaphore",
                            "outs": [],
                            "sync_info": {"on_update": [], "on_wait": [w]},
                        })
                    sync["on_wait"] = [waits[-1]]
                out.append(inst)
            blk["instructions"] = out
    return orjson.dumps(m) if changed else bir


_patched = False


def _install_waitfix():
    global _patched
    if _patched:
        return
    _patched = True
    orig = bass.Bass.to_json_bytes

    def patched(self):
        return _legalize_multiwait_json(orig(self))

    bass.Bass.to_json_bytes = patched


# ----------------------------------------------------------------------------
# Host math: psi' (state after all shared circuit parts), complex64 to track
# the reference's precision.
# ----------------------------------------------------------------------------


def _host_psi(params: np.ndarray) -> np.ndarray:
    params = np.asarray(params, np.float32)
    psi = np.zeros(DIM, np.complex64)
    psi[0] = 1.0
    for l in range(N_LAYERS):
        for q in range(N_QUBITS):
            phi, theta, lam = (np.complex64(params[l, q, i]) for i in range(3))
            rz_p = np.array([[np.exp(-0.5j * phi), 0], [0, np.exp(0.5j * phi)]],
                            np.complex64)
            rz_l = np.array([[np.exp(-0.5j * lam), 0], [0, np.exp(0.5j * lam)]],
                            np.complex64)
            c, s = np.cos(0.5 * theta), np.sin(0.5 * theta)
            ry = np.array([[c, -s], [s, c]], np.complex64)
            U = rz_l @ ry @ rz_p
            # reference einsum applies U^T
            st = psi.reshape(2 ** q, 2, -1)
            psi = np.einsum("st,lsr->ltr", U, st).astype(np.complex64).reshape(-1)
        if l < N_LAYERS - 1:
            for q in range(N_QUBITS - 1):
                st = psi.reshape(2 ** q, 2, 2, -1)
                st = np.stack([st[:, 0], np.flip(st[:, 1], axis=1)], axis=1)
                psi = st.reshape(-1)
    return psi


def _popcount_sign() -> np.ndarray:
    j = np.arange(DIM)
    pop = np.zeros(DIM, np.int64)
    for q in range(N_QUBITS):
        pop += (j >> q) & 1
    return np.where(pop % 2 == 0, 1.0, -1.0).astype(np.float32)


def _features(X: np.ndarray) -> np.ndarray:
    """Phi[b, u] = prod_q (cos(X/2) if bit(11-q) of u is 0 else sin(X/2))."""
    c = np.cos(0.5 * X).astype(np.float32)
    s = np.sin(0.5 * X).astype(np.float32)
    phi = np.ones((B, 1), np.float32)
    for q in range(N_QUBITS):
        phi = np.stack([phi * c[:, q:q + 1], phi * s[:, q:q + 1]],
                       axis=2).reshape(B, -1)
    return phi


# ----------------------------------------------------------------------------
# Pass 1: tail states T^T = W_r^T Phi^T, fp8 DoubleRow.
# Core r computes states [512r, 512r+512) x all 4096 samples.
# ----------------------------------------------------------------------------


def _build_pass1() -> bass.Bass:
    nc = bass.Bass("TRN2", target_bir_lowering=False, debug=False,
                   num_devices=NCORES)
    # w8[p, pl, kc, i, blk, c] = plane pl of W_r[kc*256+i*128+p, 512r+blk*128+c]
    w_d = nc.dram_tensor("w8", [128, 2, KCH, 2, 4, 128], f8,
                         kind="ExternalInput").ap()
    # phi[n, p, kc, i, b] = Phi8^T[kc*256+i*128+p, n*512+b]
    phi_d = nc.dram_tensor("phi", [8, 128, KCH, 2, 512], f8,
                           kind="ExternalInput").ap()
    # t[n, pl, p, blk, b] = lamP*lamW * T^T[pl, 512r+blk*128+p, n*512+b]
    t_d = nc.dram_tensor("t", [8, 2, 128, 4, 512], bf16,
                         kind="ExternalOutput").ap()

    with tile.TileContext(nc) as tc:
        with (
            tc.tile_pool(name="wpool", bufs=1) as wpool,
            tc.tile_pool(name="ppool", bufs=2) as phipool,
            tc.tile_pool(name="spool", bufs=2) as spool,
            tc.tile_pool(name="psum", bufs=1, space="PSUM") as psum,
        ):
            w8 = wpool.tile([128, 2, KCH, 2, 4, 128], f8, tag="w8")

            for n in range(8):
                phi = phipool.tile([128, KCH, 2, 512], f8, tag="phi")
                if n == 0:
                    # opening stream: 2-kchunk pieces (transfer ~ DGE setup
                    # cost) interleaved so the first matmuls start after ~2
                    # pieces instead of three full-tile transfers
                    for h in range(KCH // 2):
                        nc.sync.dma_start(phi[:, 2 * h:2 * h + 2],
                                          phi_d[0, :, 2 * h:2 * h + 2])
                        nc.sync.dma_start(w8[:, 0, 2 * h:2 * h + 2],
                                          w_d[:, 0, 2 * h:2 * h + 2])
                    for h in range(KCH // 2):
                        nc.sync.dma_start(w8[:, 1, 2 * h:2 * h + 2],
                                          w_d[:, 1, 2 * h:2 * h + 2])
                else:
                    nc.sync.dma_start(phi[:], phi_d[n])
                for pl in range(2):
                    st = spool.tile([128, 4, 512], bf16, tag=f"st{pl}",
                                    name=f"st_{n}_{pl}")
                    for blk in range(4):
                        ps = psum.tile([128, 512], f32, tag=f"ps{pl}{blk}",
                                       name=f"ps_{n}_{pl}_{blk}")
                        for k in range(KCH):
                            nc.tensor.matmul(
                                ps[:], w8[:, pl, k, :, blk, :], phi[:, k],
                                start=(k == 0), stop=(k == KCH - 1),
                                perf_mode=mybir.MatmulPerfMode.DoubleRow)
                        # gpsimd cannot access PSUM; alternate DVE/ACT
                        if blk % 2 == 0:
                            nc.vector.tensor_copy(st[:, blk], ps[:])
                        else:
                            nc.scalar.copy(st[:, blk], ps[:])
                    # one batched store per (n, pl); spread issue queues
                    (nc.sync if pl == 0 else nc.scalar).dma_start(
                        t_d[n, pl], st[:])
    return nc


# ----------------------------------------------------------------------------
# Pass 2: Karatsuba Gram + norm-corrected |.|^2, fp8 DoubleRow.
# ----------------------------------------------------------------------------


def _build_pass2() -> bass.Bass:
    nc = bass.Bass("TRN2", target_bir_lowering=False, debug=False,
                   num_devices=NCORES)
    # mv[p, pl, kc, i, f]: planes (A, B, P=A+B) of own rows (moving operand)
    mv_d = nc.dram_tensor("mv8", [128, 3, KCH, 2, BLK], f8,
                          kind="ExternalInput").ap()
    # wt[n, p, pl, kc, i, c]: planes (A, B, M=A-B) of col block n (stationary)
    wt_d = nc.dram_tensor("wt8", [NBLK, 128, 3, KCH, 2, 128], f8,
                          kind="ExternalInput").ap()
    sig_d = nc.dram_tensor("sig", [128, NBLK], f32, kind="ExternalInput").ap()
    wrow_d = nc.dram_tensor("wrow", [1, BLK], f32, kind="ExternalInput").ap()
    # ko[g, p, j, f]: block n = 4g+j -> K[row 512r+f, col block n, col p].T
    ko_d = nc.dram_tensor("ko", [NBLK // 4, 128, 4, BLK], bf16,
                          kind="ExternalOutput").ap()

    with tile.TileContext(nc) as tc:
        with (
            tc.tile_pool(name="mv", bufs=1) as mpool,
            tc.tile_pool(name="wt", bufs=3) as wpool,
            tc.tile_pool(name="dwt", bufs=1) as dpool,
            tc.tile_pool(name="post", bufs=2) as qpool,
            tc.tile_pool(name="psum", bufs=2, space="PSUM") as ppool,
        ):
            sig = mpool.tile([128, NBLK], f32, tag="sig")
            nc.sync.dma_start(sig[:], sig_d)
            wrow = mpool.tile([128, BLK], f32, tag="wrow")
            nc.sync.dma_start(wrow[:], wrow_d[0].partition_broadcast(128))

            mv = mpool.tile([128, 3, KCH, 2, BLK], f8, tag="mv")
            wt0 = wpool.tile([128, 3, KCH, 2, 128], f8, tag="wt", name="wt_0")
            # fine-grained opening: block 0's operands stream plane-by-plane
            # (wt, ACT queue) and quarter-by-quarter (mv, SP queue) so its
            # matmuls start early
            for pl in range(3):
                nc.scalar.dma_start(wt0[:, pl], wt_d[0, :, pl])
                for q in range(4):
                    nc.sync.dma_start(mv[:, pl, 4 * q:4 * q + 4],
                                      mv_d[:, pl, 4 * q:4 * q + 4])

            # blocks 16..19 are this core's diagonal cols = its own rows:
            # planes A, B are copied in-SBUF from mv (saves wt DMA); only the
            # (A-B) plane is loaded. Emitted early: they run during the
            # off-diagonal blocks when the copy engines have slack.
            def _copy(eng, dst, src):
                if eng is nc.scalar:
                    nc.scalar.copy(dst, src)
                else:
                    eng.tensor_copy(dst, src)

            dwts = []
            for d in range(4):
                dwt = dpool.tile([128, 3, KCH, 2, 128], f8, tag=f"dwt{d}")
                csl = slice(128 * d, 128 * (d + 1))
                _copy((nc.gpsimd, nc.scalar, nc.vector, nc.gpsimd)[d],
                      dwt[:, 0], mv[:, 0, :, :, csl])
                _copy((nc.scalar, nc.vector, nc.gpsimd, nc.scalar)[d],
                      dwt[:, 1], mv[:, 1, :, :, csl])
                nc.sync.dma_start(dwt[:, 2], wt_d[16 + d, :, 2])
                dwts.append(dwt)

            for n in range(NBLK):
                if n == 0:
                    wt = wt0
                elif n >= 16:
                    wt = dwts[n - 16]
                else:
                    wt = wpool.tile([128, 3, KCH, 2, 128], f8, tag="wt",
                                    name=f"wt_{n}")
                    nc.scalar.dma_start(wt[:], wt_d[n])

                ms = []
                for prod in range(3):
                    ps = ppool.tile([128, BLK], f32, tag=f"m{prod}",
                                    name=f"m{prod}_{n}")
                    for k in range(KCH):
                        nc.tensor.matmul(
                            ps[:], wt[:, prod, k], mv[:, prod, k],
                            start=(k == 0), stop=(k == KCH - 1),
                            perf_mode=mybir.MatmulPerfMode.DoubleRow)
                    ms.append(ps)
                m1, m2, m3 = ms

                # only one PSUM operand allowed per instruction
                c2 = qpool.tile([128, BLK], f32, tag="c2")
                nc.scalar.copy(c2[:], m2[:])
                t1 = qpool.tile([128, BLK], f32, tag="t1")
                nc.vector.tensor_tensor(t1[:], m1[:], c2[:],
                                        mybir.AluOpType.add)
                t2 = qpool.tile([128, BLK], f32, tag="t2")
                nc.vector.tensor_tensor(t2[:], m1[:], c2[:],
                                        mybir.AluOpType.subtract)
                t3 = qpool.tile([128, BLK], f32, tag="t3")
                # gpsimd cannot access PSUM -> DVE for the m3 read
                nc.vector.scalar_tensor_tensor(t3[:], m3[:], -1.0, t2[:],
                                               mybir.AluOpType.mult,
                                               mybir.AluOpType.add)
                sq1 = qpool.tile([128, BLK], f32, tag="sq1")
                nc.scalar.activation(sq1[:], t1[:],
                                     mybir.ActivationFunctionType.Square,
                                     scale=sig[:, n:n + 1])
                sq3 = qpool.tile([128, BLK], f32, tag="sq3")
                nc.scalar.activation(sq3[:], t3[:],
                                     mybir.ActivationFunctionType.Square,
                                     scale=sig[:, n:n + 1])
                ss = qpool.tile([128, BLK], f32, tag="ss")
                nc.gpsimd.tensor_tensor(ss[:], sq1[:], sq3[:],
                                        mybir.AluOpType.add)
                if n % 4 == 0:
                    kos = qpool.tile([128, 4, BLK], bf16, tag="kos",
                                     name=f"kos_{n // 4}")
                nc.vector.tensor_tensor(kos[:, n % 4], ss[:], wrow[:],
                                        mybir.AluOpType.mult)
                if n % 4 == 3:
                    # one batched store per 4 blocks; SP queue has slack
                    nc.sync.dma_start(ko_d[n // 4], kos[:])
    return nc


_nc1 = None
_nc2 = None

PROFILE = False
LAST_PROFILE: dict = {}


def kernel(X: np.ndarray, params: np.ndarray) -> np.ndarray:
    global _nc1, _nc2
    _install_waitfix()
    X = np.asarray(X, np.float32)
    params = np.asarray(params, np.float32)

    # ---- host precompute -------------------------------------------------
    psi = _host_psi(params)
    psi0 = psi[0]
    r = psi.copy()
    r[0] = 0.0
    sgn = _popcount_sign()
    phi = _features(X)                       # (B, DIM) f32

    jj = np.arange(DIM)
    XORm = np.bitwise_xor.outer(jj, jj)      # (u, j)
    ANDm = np.bitwise_and.outer(jj, jj)
    sgn_uj = sgn[ANDm]
    w_re = sgn_uj * r.real[XORm]
    w_im = sgn_uj * r.imag[XORm]
    lam_w = float(224.0 / max(np.abs(w_re).max(), np.abs(w_im).max(), 1e-30))
    w8 = np.stack([(w_re * lam_w).astype(npf8),
                   (w_im * lam_w).astype(npf8)])      # (2, DIM u, DIM j)
    lam_p = 64.0
    phi8t = np.ascontiguousarray((phi.T * lam_p).astype(npf8))   # (u, b)

    # per-core pass-1 inputs
    phi_in = np.ascontiguousarray(
        phi8t.reshape(KCH, 2, 128, 8, 512).transpose(3, 2, 0, 1, 4))
    in_maps1 = []
    for cr in range(NCORES):
        wc = w8[:, :, cr * BLK:(cr + 1) * BLK]        # (2, DIM, 512)
        wc = wc.reshape(2, KCH, 2, 128, 4, 128).transpose(3, 0, 1, 2, 4, 5)
        in_maps1.append({"w8": np.ascontiguousarray(wc), "phi": phi_in})

    if _nc1 is None:
        _nc1 = _build_pass1()
    res1 = run_bass_kernel_spmd(_nc1, in_maps1, core_ids=list(range(NCORES)))

    # ---- host mid: assemble S, quantize planes ---------------------------
    inv_lw = 1.0 / (lam_p * lam_w)
    phiT = phi.T                                      # (j, b)
    A = np.empty((DIM, B), np.float32)
    Bp = np.empty((DIM, B), np.float32)
    for cr in range(NCORES):
        t = res1.results[cr]["t"].astype(np.float32) * inv_lw  # (8,2,128,4,512)
        rows = slice(cr * BLK, (cr + 1) * BLK)
        tt = t.transpose(1, 3, 2, 0, 4).reshape(2, BLK, B)
        A[rows] = tt[0]
        Bp[rows] = tt[1]
    A += psi0.real * sgn[:, None] * phiT
    Bp += psi0.imag * sgn[:, None] * phiT
    nrm = np.sqrt(np.einsum("jb,jb->b", A, A) + np.einsum("jb,jb->b", Bp, Bp))
    A *= (1.0 / nrm)[None, :]
    Bp *= (1.0 / nrm)[None, :]

    A8 = (A * LAM).astype(npf8)
    B8 = (Bp * LAM).astype(npf8)
    A8f = A8.astype(np.float32)
    B8f = B8.astype(np.float32)
    P8 = (A8f + B8f).astype(npf8)
    M8 = (A8f - B8f).astype(npf8)
    rho2 = (np.einsum("jb,jb->b", A8f, A8f)
            + np.einsum("jb,jb->b", B8f, B8f)) / (LAM * LAM)    # (B,)

    pl_mv = np.stack([A8, B8, P8])    # (3, j, b)
    pl_wt = np.stack([A8, B8, M8])
    sig_all = (1.0 / (LAM * LAM * np.sqrt(rho2))).astype(np.float32)
    wrow_all = (1.0 / rho2).astype(np.float32)

    # strip layout: 16 off-diagonal col blocks first (strip offsets
    # 512..2560), the 4 diagonal blocks (offsets 0..512) last — the device
    # fills the diagonal stationary tiles by SBUF copies from mv.
    colrel = np.concatenate([np.arange(BLK, NB_COLS), np.arange(0, BLK)])
    in_maps2 = []
    for cr in range(NCORES):
        cols = (cr * BLK + colrel) % B
        mvc = pl_mv[:, :, cr * BLK:(cr + 1) * BLK]    # (3, DIM, 512)
        mvc = mvc.reshape(3, KCH, 2, 128, BLK).transpose(3, 0, 1, 2, 4)
        wtc = pl_wt[:, :, cols]                       # (3, DIM, 2560)
        wtc = (wtc.reshape(3, KCH, 2, 128, NBLK, 128)
               .transpose(4, 3, 0, 1, 2, 5))
        sig = sig_all[cols].reshape(NBLK, 128).T      # (128, NBLK)
        wrow = wrow_all[cr * BLK:(cr + 1) * BLK][None, :]
        in_maps2.append({
            "mv8": np.ascontiguousarray(mvc),
            "wt8": np.ascontiguousarray(wtc),
            "sig": np.ascontiguousarray(sig),
            "wrow": np.ascontiguousarray(wrow),
        })

    if _nc2 is None:
        _nc2 = _build_pass2()
    res2 = run_bass_kernel_spmd(_nc2, in_maps2, core_ids=list(range(NCORES)))

    # ---- assemble K (with symmetric mirroring) ---------------------------
    K = np.empty((B, B), np.float32)
    for cr in range(NCORES):
        # (NBLK//4, 128, 4, BLK) -> (NBLK, 128, BLK)
        ko = (res2.results[cr]["ko"].astype(np.float32)
              .transpose(0, 2, 1, 3).reshape(NBLK, 128, BLK))
        rows = slice(cr * BLK, (cr + 1) * BLK)
        for n in range(NBLK):
            gs = (cr * BLK + int(colrel[n * 128])) % B
            colsl = slice(gs, gs + 128)
            K[rows, colsl] = ko[n].T
            d = 1 + n // 4 if n < 16 else 0
            if 0 < d < 4 or (d == 4 and cr < 4):
                K[colsl, rows] = ko[n]
    return K
